# revision 12
# baseline (speedup 1.0000x reference)
"""2-layer GAT on 8 Trainium2 NeuronCores (Bass/Tile).

Sharding: edges sorted by destination node; nodes partitioned 8 x N/8 across
cores (dst-partitioned edge-parallel). Per-dst softmax groups stay entirely on
one core, so aggregation needs no cross-core reduction. Per 128-node block:
dma_gather of per-edge source feature rows from a replicated table (two halves
so indices fit int16), one-hot S_T built via is_equal, and the weighted
scatter-add + softmax denominator computed as ONE PSUM-accumulated bf16 matmul
per 128-edge tile (weights written into the gathered rows' score columns).
One AllGather shares the small layer-2 feature table between the layers.
"""
import numpy as np

P = 128
NCORES = 8

_CACHE = {}


def _wrap_idx_segments(segs, total_cols):
    arr = np.zeros((16, total_cols), np.int16)
    for off, idx in segs:
        n = len(idx)
        if n:
            arr[:, off:off + n // 16] = idx.reshape(n // 16, 16).T
    return np.tile(arr, (8, 1))


def _prep(x, edge_index):
    N = x.shape[0]
    NPC = N // NCORES
    NB = (NPC + P - 1) // P
    SPLIT = N // 2

    src = np.concatenate([np.asarray(edge_index[0]), np.arange(N, dtype=np.int64)])
    dst = np.concatenate([np.asarray(edge_index[1]), np.arange(N, dtype=np.int64)])
    order = np.argsort(dst, kind="stable")
    s_all = src[order].astype(np.int64)
    d_all = dst[order].astype(np.int64)

    lists = [[[None, None] for _ in range(NB)] for _ in range(NCORES)]
    for c in range(NCORES):
        base = c * NPC
        for b in range(NB):
            e0 = np.searchsorted(d_all, base + b * P)
            e1 = np.searchsorted(d_all, min(base + (b + 1) * P, base + NPC))
            ss, dd = s_all[e0:e1], d_all[e0:e1]
            m = ss < SPLIT
            lists[c][b][0] = (ss[m], dd[m])
            lists[c][b][1] = (ss[~m], dd[~m])

    NT = np.zeros((NB, 2), np.int64)
    for b in range(NB):
        for s in range(2):
            mx = max(len(lists[c][b][s][0]) for c in range(NCORES))
            NT[b, s] = (mx + P - 1) // P

    groups = [list(range(g, min(g + 2, NB))) for g in range(0, NB, 2)]

    tile_of = np.zeros((NB, 2), np.int64)
    t = 0
    for b in range(NB):
        for s in range(2):
            tile_of[b, s] = t
            t += int(NT[b, s])
    NTOT = t

    g_cols, g_off = 0, []
    for g, blocks in enumerate(groups):
        offs = []
        for s in range(2):
            ntg = int(sum(NT[b, s] for b in blocks))
            offs.append((g_cols, ntg))
            g_cols += ntg * 8
        g_off.append(offs)
    d_cols, d_off = 0, []
    for g, blocks in enumerate(groups):
        ntg = int(sum(NT[b, 0] + NT[b, 1] for b in blocks))
        d_off.append((d_cols, ntg))
        d_cols += ntg * 8

    plan = dict(N=N, NPC=NPC, NB=NB, SPLIT=SPLIT, NT=NT, groups=groups,
                tile_of=tile_of, NTOT=NTOT, g_off=g_off, d_off=d_off,
                g_cols=g_cols, d_cols=d_cols)

    per_core = []
    for c in range(NCORES):
        base = c * NPC
        gsegs, dsegs = [], []
        d_fp = np.full((NTOT, P), -1.0, np.float32)
        for g, blocks in enumerate(groups):
            for s in range(2):
                col0, ntg = g_off[g][s]
                idx = np.zeros(ntg * P, np.int64)
                pos = 0
                for b in blocks:
                    ss = lists[c][b][s][0]
                    nslots = int(NT[b, s]) * P
                    idx[pos:pos + len(ss)] = ss - (SPLIT if s == 1 else 0)
                    pos += nslots
                gsegs.append((col0, idx.astype(np.int16)))
            col0, ntg = d_off[g]
            didx = np.zeros(ntg * P, np.int64)
            pos = 0
            for b in blocks:
                for s in range(2):
                    ss, dd = lists[c][b][s]
                    nslots = int(NT[b, s]) * P
                    didx[pos:pos + len(dd)] = dd - base
                    pos += nslots
                    t0 = int(tile_of[b, s])
                    dv = np.full(nslots, -1.0, np.float32)
                    dv[:len(dd)] = (dd - base - b * P).astype(np.float32)
                    d_fp[t0:t0 + int(NT[b, s])] = dv.reshape(int(NT[b, s]), P)
            dsegs.append((col0, didx.astype(np.int16)))
        per_core.append(dict(
            g_idx=_wrap_idx_segments(gsegs, g_cols),
            dl_idx=_wrap_idx_segments(dsegs, d_cols),
            d_fpT=np.ascontiguousarray(d_fp.T),   # [P, NTOT]
        ))
    return plan, per_core


def _build(plan, dims, has_b1, has_b2):
    import os
    V_ST = os.environ.get("GAT_V_ST", "merged")      # merged | base
    V_EXP = os.environ.get("GAT_V_EXP", "strided")   # strided | wbuf
    V_GATHER = os.environ.get("GAT_V_GATHER", "merged")  # merged | chunk8
    import concourse.bass as bass
    import concourse.bacc as bacc
    import concourse.tile as tile
    from concourse import mybir

    f32 = mybir.dt.float32
    bf16 = mybir.dt.bfloat16
    i16 = mybir.dt.int16
    AF = mybir.ActivationFunctionType
    OP = mybir.AluOpType

    N, NPC, NB, SPLIT = plan["N"], plan["NPC"], plan["NB"], plan["SPLIT"]
    NT, groups, tile_of, NTOT = (plan["NT"], plan["groups"], plan["tile_of"],
                                 plan["NTOT"])
    HID, H1, C1, OUT = dims["HID"], dims["H1"], dims["C1"], dims["OUT"]
    NPAD = ((N + P - 1) // P) * P
    NBA = NPAD // P
    ROW1 = 384                 # hext1 row stride (768B: mult of 256B)
    W1C = HID + 2 * H1         # 272 useful cols of hext1
    ROW2 = 128                 # 256B rows for score/L2 tables
    W2C = OUT + 2
    NBLK = NB * P
    NEG = 0.2

    nc = bacc.Bacc(num_devices=NCORES, num_swdge_queues=4)

    xT = nc.dram_tensor("xT", [2, P, NPAD], bf16, kind="ExternalInput")
    xTo = nc.dram_tensor("xTo", [2, P, NBLK], bf16, kind="ExternalInput")
    w1e = nc.dram_tensor("w1e", [2, P, W1C], bf16, kind="ExternalInput")
    w1s = nc.dram_tensor("w1s", [2, P, 2 * H1], bf16, kind="ExternalInput")
    w2e = nc.dram_tensor("w2e", [2, P, W2C], bf16, kind="ExternalInput")
    negcs = nc.dram_tensor("negcs", [P, W2C], f32, kind="ExternalInput")
    g_idx_d = nc.dram_tensor("g_idx", [P, plan["g_cols"]], i16, kind="ExternalInput")
    dl_idx_d = nc.dram_tensor("dl_idx", [P, plan["d_cols"]], i16, kind="ExternalInput")
    d_fpT_d = nc.dram_tensor("d_fpT", [P, NTOT], f32, kind="ExternalInput")
    if has_b1:
        b1_d = nc.dram_tensor("b1r", [P, HID], bf16, kind="ExternalInput")
    if has_b2:
        b2_d = nc.dram_tensor("b2r", [P, OUT], f32, kind="ExternalInput")
    out2 = nc.dram_tensor("out2", [NPC, OUT], f32, kind="ExternalOutput")

    hext1 = nc.dram_tensor("hext1", [NPAD + P, ROW1], bf16)
    s1dst = nc.dram_tensor("s1dst", [NBLK + P, ROW2], bf16)
    h2loc = nc.dram_tensor("h2loc", [NBLK + P, ROW2], bf16)
    h2full = nc.dram_tensor("h2full", [N + P, ROW2], bf16, addr_space="Shared")

    def sub_ap(t, elem_off, dims_):
        a = t[:]
        return bass.AP(tensor=a.tensor, offset=a.offset + elem_off,
                       ap=[a.ap[0]] + dims_)

    with tile.TileContext(nc, num_cores=NCORES) as tc:
        with tc.tile_pool(name="consts", bufs=1) as cp:
            w1t = []
            w1st = []
            w2t = []
            for k in range(2):
                t1 = cp.tile([P, W1C], bf16, tag=f"w1t{k}")
                nc.sync.dma_start(out=t1[:], in_=w1e[k])
                w1t.append(t1)
                t2 = cp.tile([P, 2 * H1], bf16, tag=f"w1st{k}")
                nc.sync.dma_start(out=t2[:], in_=w1s[k])
                w1st.append(t2)
                t3 = cp.tile([P, W2C], bf16, tag=f"w2t{k}")
                nc.sync.dma_start(out=t3[:], in_=w2e[k])
                w2t.append(t3)
            ncs_t = cp.tile([P, W2C], f32)
            nc.sync.dma_start(out=ncs_t[:], in_=negcs[:])
            gidx_t = cp.tile([P, plan["g_cols"]], i16)
            nc.sync.dma_start(out=gidx_t[:], in_=g_idx_d[:])
            dlidx_t = cp.tile([P, plan["d_cols"]], i16)
            nc.sync.dma_start(out=dlidx_t[:], in_=dl_idx_d[:])
            dfp_t = cp.tile([P, NTOT], f32)
            nc.sync.dma_start(out=dfp_t[:], in_=d_fpT_d[:])
            iota_i = cp.tile([P, P], mybir.dt.int32)
            nc.gpsimd.iota(iota_i[:], pattern=[[1, P]], base=0,
                           channel_multiplier=0)
            iota_t = cp.tile([P, P], bf16)
            nc.vector.tensor_copy(out=iota_t[:], in_=iota_i[:])
            iota_f = cp.tile([P, P], f32)
            nc.vector.tensor_copy(out=iota_f[:], in_=iota_i[:])
            pidx_i = cp.tile([P, 1], mybir.dt.int32)
            nc.gpsimd.iota(pidx_i[:], pattern=[[0, 1]], base=0,
                           channel_multiplier=1)
            pidx_f = cp.tile([P, 1], f32)
            nc.vector.tensor_copy(out=pidx_f[:], in_=pidx_i[:])
            ident = cp.tile([P, P], bf16)
            nc.vector.tensor_scalar(out=ident[:], in0=iota_t[:], scalar1=pidx_f[:],
                                    scalar2=None, op0=OP.is_equal)
            b1_t = b2_t = None
            if has_b1:
                b1_t = cp.tile([P, HID], bf16)
                nc.sync.dma_start(out=b1_t[:], in_=b1_d[:])
            if has_b2:
                b2_t = cp.tile([P, OUT], f32)
                nc.sync.dma_start(out=b2_t[:], in_=b2_d[:])

            # ---------------- phase A: hext1 = [x@W1 | x@W1@A] ----------------
            CH = 8
            with (
                tc.tile_pool(name="xc", bufs=4) as xc,
                tc.tile_pool(name="psA", bufs=4, space="PSUM") as psA,
                tc.tile_pool(name="rowp", bufs=6) as rowp,
            ):
                for ch in range(0, NBA, CH):
                    ntc = min(CH, NBA - ch)
                    ck = []
                    for kh in range(2):
                        t_ = xc.tile([P, CH * P], bf16, tag="xchunk")
                        nc.sync.dma_start(out=t_[:, :ntc * P],
                                          in_=xT[kh, :, ch * P:(ch + ntc) * P])
                        ck.append(t_)
                    for j in range(ntc):
                        i = ch + j
                        ps = psA.tile([P, W1C], f32, tag="psA")
                        nc.tensor.matmul(ps[:], ck[0][:, j * P:(j + 1) * P],
                                         w1t[0][:], start=True, stop=False)
                        nc.tensor.matmul(ps[:], ck[1][:, j * P:(j + 1) * P],
                                         w1t[1][:], start=False, stop=True)
                        row = rowp.tile([P, W1C], bf16, tag="row")
                        if i % 2 == 0:
                            nc.scalar.activation(row[:], ps[:], AF.Copy)
                        else:
                            nc.vector.tensor_copy(out=row[:], in_=ps[:])
                        nc.sync.dma_start(out=hext1[i * P:(i + 1) * P, 0:W1C],
                                          in_=row[:])
                for ch in range(0, NB, CH):
                    ntc = min(CH, NB - ch)
                    ck = []
                    for kh in range(2):
                        t_ = xc.tile([P, CH * P], bf16, tag="xchunk2")
                        nc.sync.dma_start(out=t_[:, :ntc * P],
                                          in_=xTo[kh, :, ch * P:(ch + ntc) * P])
                        ck.append(t_)
                    for j in range(ntc):
                        i = ch + j
                        ps = psA.tile([P, 2 * H1], f32, tag="psA2")
                        nc.tensor.matmul(ps[:], ck[0][:, j * P:(j + 1) * P],
                                         w1st[0][:], start=True, stop=False)
                        nc.tensor.matmul(ps[:], ck[1][:, j * P:(j + 1) * P],
                                         w1st[1][:], start=False, stop=True)
                        row = rowp.tile([P, 2 * H1], bf16, tag="rows")
                        nc.vector.tensor_copy(out=row[:], in_=ps[:])
                        nc.sync.dma_start(out=s1dst[i * P:(i + 1) * P, 0:2 * H1],
                                          in_=row[:])

            # ---------------- GAT conv layers ----------------
            def layer(lidx, table, nrows_tab, srow_g, tdst, H, F, sc_src, sc_dst,
                      epilogue):
                # gathered src row: [0:F feats | sc_src: H src scores]; the H
                # edge weights are written back over the src-score columns so
                # numerator and denominator come from ONE matmul per tile.
                MC = F + H
                qn = [0]
                with (
                    tc.tile_pool(name=f"g{lidx}", bufs=2) as gp,
                    tc.tile_pool(name=f"gd{lidx}", bufs=2) as gdp,
                    tc.tile_pool(name=f"wk{lidx}", bufs=3) as wk,
                    tc.tile_pool(name=f"st{lidx}", bufs=2) as stp,
                    tc.tile_pool(name=f"ps{lidx}", bufs=2, space="PSUM") as psp,
                    tc.tile_pool(name=f"pse{lidx}", bufs=2, space="PSUM") as pse,
                    tc.tile_pool(name=f"ep{lidx}", bufs=3) as ep,
                ):
                    for g, blocks in enumerate(groups):
                        gbuf = [None, None]
                        for s in range(2):
                            col0, ntg = plan["g_off"][g][s]
                            if ntg == 0:
                                continue
                            gt = gp.tile([P, ntg, srow_g], bf16, tag=f"g{s}")
                            base = 0 if s == 0 else SPLIT * ROW_TAB[lidx]
                            inap = bass.AP(
                                tensor=table[:].tensor, offset=base,
                                ap=[[ROW_TAB[lidx],
                                     nrows_tab - (0 if s == 0 else SPLIT)],
                                    [1, srow_g]])
                            cstep = ntg if V_GATHER == "merged" else 8
                            for c0 in range(0, ntg, cstep):
                                cn = min(cstep, ntg - c0)
                                nc.gpsimd.dma_gather(
                                    gt[:, c0:c0 + cn, :], inap,
                                    gidx_t[:, col0 + c0 * 8:col0 + (c0 + cn) * 8],
                                    cn * P, cn * P, srow_g,
                                    elem_step=ROW_TAB[lidx],
                                    single_packet=(cn <= 8),
                                    queue_num=qn[0] % 4)
                                qn[0] += 1
                            gbuf[s] = gt
                        dcol0, dntg = plan["d_off"][g]
                        gdt = gdp.tile([P, dntg, ROW2], bf16, tag="gd")
                        cstep = dntg if V_GATHER == "merged" else 8
                        for c0 in range(0, dntg, cstep):
                            cn = min(cstep, dntg - c0)
                            nc.gpsimd.dma_gather(
                                gdt[:, c0:c0 + cn, :], tdst[:],
                                dlidx_t[:, dcol0 + c0 * 8:dcol0 + (c0 + cn) * 8],
                                cn * P, cn * P, ROW2, elem_step=ROW2,
                                single_packet=(cn <= 8),
                                queue_num=qn[0] % 4)
                            qn[0] += 1

                        goff = [0, 0]
                        doff = 0
                        for b in blocks:
                            ntb = int(NT[b, 0] + NT[b, 1])
                            if ntb == 0:
                                continue
                            t0 = int(tile_of[b, 0])
                            # one-hot S_T for all tiles of the block
                            st = stp.tile([P, ntb, P], bf16, tag="st")
                            if V_ST == "merged":
                                in0 = sub_ap(iota_f, 0, [[0, ntb], [1, P]])
                                in1 = sub_ap(dfp_t, t0, [[1, ntb], [0, P]])
                                nc.vector.tensor_tensor(out=st[:], in0=in0,
                                                        in1=in1, op=OP.is_equal)
                            else:
                                for ti_ in range(ntb):
                                    nc.vector.tensor_scalar(
                                        out=st[:, ti_, :], in0=iota_t[:],
                                        scalar1=dfp_t[:, t0 + ti_:t0 + ti_ + 1],
                                        scalar2=None, op0=OP.is_equal)
                            # alpha = s_src[src] + s_dst[dst]  (f32)
                            al = wk.tile([P, ntb * H], f32, tag="al")
                            toff = 0
                            for s in range(2):
                                nts = int(NT[b, s])
                                if nts == 0:
                                    continue
                                gt = gbuf[s]
                                src_ap = sub_ap(gt, goff[s] * srow_g + sc_src,
                                                [[srow_g, nts], [1, H]])
                                dst_ap = sub_ap(gdt, (doff + toff) * ROW2 + sc_dst,
                                                [[ROW2, nts], [1, H]])
                                out_ap = sub_ap(al, toff * H,
                                                [[H, nts], [1, H]])
                                nc.vector.tensor_tensor(out=out_ap, in0=src_ap,
                                                        in1=dst_ap, op=OP.add)
                                toff += nts
                            # leaky relu in f32
                            al2 = wk.tile([P, ntb * H], f32, tag="al2")
                            nc.vector.tensor_scalar(out=al2[:], in0=al[:],
                                                    scalar1=NEG, scalar2=None,
                                                    op0=OP.mult)
                            nc.vector.tensor_tensor(out=al[:], in0=al[:],
                                                    in1=al2[:], op=OP.max)
                            # w = exp(alpha) written over the src-score cols,
                            # then weight the gathered features in place
                            if V_EXP == "wbuf":
                                wbuf = wk.tile([P, ntb * H], bf16, tag="wb")
                                nc.scalar.activation(wbuf[:], al[:], AF.Exp)
                            toff = 0
                            for s in range(2):
                                nts = int(NT[b, s])
                                if nts == 0:
                                    continue
                                gt = gbuf[s]
                                w_ap = sub_ap(gt, goff[s] * srow_g + sc_src,
                                              [[srow_g, nts], [1, H]])
                                al_ap = sub_ap(al, toff * H,
                                               [[1, nts * H]])
                                if V_EXP == "wbuf":
                                    wb_ap = sub_ap(wbuf, toff * H,
                                                   [[H, nts], [1, H]])
                                    nc.vector.tensor_copy(out=w_ap, in_=wb_ap)
                                else:
                                    nc.scalar.activation(w_ap, al_ap, AF.Exp)
                                if H > 1:
                                    gv = sub_ap(gt, goff[s] * srow_g,
                                                [[srow_g, nts], [F // H, H],
                                                 [1, F // H]])
                                    win = sub_ap(gt, goff[s] * srow_g + sc_src,
                                                 [[srow_g, nts], [1, H],
                                                  [0, F // H]])
                                else:
                                    gv = sub_ap(gt, goff[s] * srow_g,
                                                [[srow_g, nts], [1, F]])
                                    win = sub_ap(gt, goff[s] * srow_g + sc_src,
                                                 [[srow_g, nts], [0, F]])
                                nc.vector.tensor_tensor(out=gv, in0=gv, in1=win,
                                                        op=OP.mult)
                                toff += nts
                            # fused numerator+denominator matmul chain
                            ps = psp.tile([P, MC], f32, tag="num")
                            ti = 0
                            for s in range(2):
                                nts = int(NT[b, s])
                                gt = gbuf[s]
                                for j in range(nts):
                                    nc.tensor.matmul(
                                        ps[:], st[:, ti, :],
                                        gt[:, goff[s] + j, 0:MC],
                                        start=(ti == 0), stop=(ti == ntb - 1))
                                    ti += 1
                            rows = min(P, NPC - b * P)
                            epilogue(b, rows, ps, ep, pse)
                            goff[0] += int(NT[b, 0])
                            goff[1] += int(NT[b, 1])
                            doff += ntb

            def epi1(b, rows, ps, ep, pse):
                rden = ep.tile([P, H1], f32, tag="rden")
                nc.vector.reciprocal(rden[:], ps[:, HID:HID + H1])
                o = ep.tile([P, HID], bf16, tag="o")
                rb = sub_ap(rden, 0, [[1, H1], [0, C1]])
                num2 = bass.AP(tensor=ps[:].tensor, offset=ps[:].offset,
                               ap=[ps[:].ap[0], [C1, H1], [1, C1]])
                o2d = bass.AP(tensor=o[:].tensor, offset=o[:].offset,
                              ap=[o[:].ap[0], [C1, H1], [1, C1]])
                nc.vector.tensor_tensor(out=o2d, in0=num2, in1=rb, op=OP.mult)
                if b1_t is not None:
                    nc.vector.tensor_tensor(out=o[:], in0=o[:], in1=b1_t[:],
                                            op=OP.add)
                e = ep.tile([P, HID], bf16, tag="e")
                nc.scalar.activation(e[:], o[:], AF.Exp)
                nc.vector.tensor_scalar(out=o[:], in0=o[:], scalar1=0.0,
                                        scalar2=None, op0=OP.max)
                nc.vector.tensor_scalar(out=e[:], in0=e[:], scalar1=1.0,
                                        scalar2=None, op0=OP.min)
                nc.vector.tensor_tensor(out=o[:], in0=o[:], in1=e[:], op=OP.add)
                h2ps = pse.tile([P, W2C], f32, tag="h2ps")
                for half in range(2):
                    pt = pse.tile([P, P], bf16, tag="pt")
                    nc.tensor.transpose(pt[:], o[:, half * P:(half + 1) * P],
                                        ident[:])
                    et = ep.tile([P, P], bf16, tag="et")
                    nc.vector.tensor_copy(out=et[:], in_=pt[:])
                    nc.tensor.matmul(h2ps[:], et[:], w2t[half][:],
                                     start=(half == 0), stop=(half == 1))
                h2row = ep.tile([P, ROW2], bf16, tag="h2row")
                nc.vector.tensor_tensor(out=h2row[:, 0:W2C], in0=h2ps[:],
                                        in1=ncs_t[:], op=OP.add)
                nc.sync.dma_start(out=h2loc[b * P:b * P + P, :], in_=h2row[:])

            def epi2(b, rows, ps, ep, pse):
                rden = ep.tile([P, 1], f32, tag="rden2")
                nc.vector.reciprocal(rden[:], ps[:, OUT:OUT + 1])
                o = ep.tile([P, OUT], f32, tag="o2")
                nc.vector.tensor_scalar(out=o[:], in0=ps[:, 0:OUT],
                                        scalar1=rden[:],
                                        scalar2=None, op0=OP.mult)
                if b2_t is not None:
                    nc.vector.tensor_tensor(out=o[:], in0=o[:], in1=b2_t[:],
                                            op=OP.add)
                nc.sync.dma_start(out=out2[b * P:b * P + rows, :],
                                  in_=o[:rows, :])

            ROW_TAB = {1: ROW1, 2: ROW2}
            layer(1, hext1, NPAD, ROW1, s1dst, H1, HID, HID, H1, epi1)
            nc.gpsimd.collective_compute(
                "AllGather", mybir.AluOpType.bypass,
                replica_groups=[list(range(NCORES))],
                ins=[h2loc[0:NPC, :]], outs=[h2full[0:N, :]],
            )
            layer(2, h2full, N, ROW2, h2loc, 1, OUT, OUT, OUT + 1, epi2)

    nc.finalize()
    return nc


def _host_prep_weights(W1, att1, W2, att2):
    HID = W1.shape[1]
    H1 = att1.shape[1]
    C1 = HID // H1
    OUT = W2.shape[1]
    A_src = np.zeros((HID, H1), np.float32)
    A_dst = np.zeros((HID, H1), np.float32)
    for h in range(H1):
        A_src[h * C1:(h + 1) * C1, h] = att1[0, h, C1:]
        A_dst[h * C1:(h + 1) * C1, h] = att1[0, h, :C1]
    W1ext = np.concatenate([W1, W1 @ A_src, W1 @ A_dst], axis=1)
    W1sco = np.concatenate([W1 @ A_src, W1 @ A_dst], axis=1)
    a2 = att2[0, 0]
    W2ext = np.concatenate([W2, (W2 @ a2[OUT:])[:, None],
                            (W2 @ a2[:OUT])[:, None]], axis=1)
    return W1ext, W1sco, W2ext


def kernel(x, edge_index, W1, att1, b1, W2, att2, b2):
    import os
    from concourse import mybir
    from concourse.bass_utils import run_bass_kernel_spmd
    ml_bf16 = mybir.dt.np(mybir.dt.bfloat16)

    x = np.asarray(x, np.float32)
    edge_index = np.asarray(edge_index)
    W1 = np.asarray(W1, np.float32)
    att1 = np.asarray(att1, np.float32)
    b1 = np.asarray(b1, np.float32)
    W2 = np.asarray(W2, np.float32)
    att2 = np.asarray(att2, np.float32)
    b2 = np.asarray(b2, np.float32)

    N, IN = x.shape
    HID = W1.shape[1]
    H1 = att1.shape[1]
    C1 = HID // H1
    OUT = W2.shape[1]
    NPC = N // NCORES
    NB = (NPC + P - 1) // P
    NPAD = ((N + P - 1) // P) * P
    NBLK = NB * P

    plan, per_core = _prep(x, edge_index)
    dims = dict(IN=IN, HID=HID, H1=H1, C1=C1, OUT=OUT)
    has_b1 = bool(np.any(b1 != 0))
    has_b2 = bool(np.any(b2 != 0))

    key = (N, IN, HID, H1, OUT, plan["g_cols"], plan["d_cols"], plan["NTOT"],
           has_b1, has_b2, tuple(int(v) for v in plan["NT"].ravel()),
           os.environ.get("GAT_V_ST"), os.environ.get("GAT_V_EXP"),
           os.environ.get("GAT_V_GATHER"))
    if key not in _CACHE:
        _CACHE[key] = _build(plan, dims, has_b1, has_b2)
    nc = _CACHE[key]

    W1ext, W1sco, W2ext = _host_prep_weights(W1, att1, W2, att2)
    negcs = np.tile(-W2ext.sum(axis=0, keepdims=True), (P, 1)).astype(np.float32)

    xTfull = np.zeros((IN, NPAD), np.float32)
    xTfull[:, :N] = x.T
    xT = xTfull.reshape(2, P, NPAD).astype(ml_bf16)

    def ktiles(w):
        return np.ascontiguousarray(w.reshape(2, P, -1)).astype(ml_bf16)

    in_maps = []
    for c in range(NCORES):
        xo = np.zeros((IN, NBLK), np.float32)
        xo[:, :NPC] = x[c * NPC:(c + 1) * NPC].T
        m = dict(
            xT=xT,
            xTo=xo.reshape(2, P, NBLK).astype(ml_bf16),
            w1e=ktiles(W1ext),
            w1s=ktiles(W1sco),
            w2e=ktiles(W2ext),
            negcs=negcs,
            g_idx=per_core[c]["g_idx"],
            dl_idx=per_core[c]["dl_idx"],
            d_fpT=per_core[c]["d_fpT"],
        )
        if has_b1:
            m["b1r"] = np.tile(b1[None, :], (P, 1)).astype(ml_bf16)
        if has_b2:
            m["b2r"] = np.tile(b2[None, :], (P, 1)).astype(np.float32)
        in_maps.append(m)

    trace = bool(os.environ.get("GAT_TRACE"))
    kw = {}
    if trace:
        kw = dict(trace=True,
                  tmpdir=os.environ.get("GAT_TRACE_DIR") or None,
                  trace_cores=[int(c) for c in
                               os.environ.get("GAT_TRACE_CORES", "0").split(",")])
    res = run_bass_kernel_spmd(nc, in_maps, list(range(NCORES)), **kw)
    globals()["LAST_RESULT"] = res
    out = np.concatenate([res.results[c]["out2"] for c in range(NCORES)], axis=0)
    return np.ascontiguousarray(out.astype(np.float32))


# revision 15
# speedup vs baseline: 1.6064x; 1.6064x over previous
"""2-layer GAT on 8 Trainium2 NeuronCores (Bass/Tile).

Sharding: edges sorted by destination node; nodes partitioned 8 x N/8 across
cores (dst-partitioned edge-parallel). Per-dst softmax groups stay entirely on
one core, so aggregation needs no cross-core reduction. Per 128-node block:
dma_gather of per-edge source feature rows from a replicated table (two halves
so indices fit int16), one-hot S_T built via is_equal, and the weighted
scatter-add + softmax denominator computed as ONE PSUM-accumulated bf16 matmul
per 128-edge tile (weights written into the gathered rows' score columns).
One AllGather shares the small layer-2 feature table between the layers.
"""
import numpy as np

P = 128
NCORES = 8

_CACHE = {}


def _wrap_idx_segments(segs, total_cols):
    arr = np.zeros((16, total_cols), np.int16)
    for off, idx in segs:
        n = len(idx)
        if n:
            arr[:, off:off + n // 16] = idx.reshape(n // 16, 16).T
    return np.tile(arr, (8, 1))


def _prep(x, edge_index):
    N = x.shape[0]
    NPC = N // NCORES
    NB = (NPC + P - 1) // P
    SPLIT = N // 2

    src = np.concatenate([np.asarray(edge_index[0]), np.arange(N, dtype=np.int64)])
    dst = np.concatenate([np.asarray(edge_index[1]), np.arange(N, dtype=np.int64)])
    order = np.argsort(dst, kind="stable")
    s_all = src[order].astype(np.int64)
    d_all = dst[order].astype(np.int64)

    lists = [[[None, None] for _ in range(NB)] for _ in range(NCORES)]
    for c in range(NCORES):
        base = c * NPC
        for b in range(NB):
            e0 = np.searchsorted(d_all, base + b * P)
            e1 = np.searchsorted(d_all, min(base + (b + 1) * P, base + NPC))
            ss, dd = s_all[e0:e1], d_all[e0:e1]
            m = ss < SPLIT
            lists[c][b][0] = (ss[m], dd[m])
            lists[c][b][1] = (ss[~m], dd[~m])

    NT = np.zeros((NB, 2), np.int64)
    for b in range(NB):
        for s in range(2):
            mx = max(len(lists[c][b][s][0]) for c in range(NCORES))
            NT[b, s] = (mx + P - 1) // P

    groups = [list(range(g, min(g + 2, NB))) for g in range(0, NB, 2)]

    tile_of = np.zeros((NB, 2), np.int64)
    t = 0
    for b in range(NB):
        for s in range(2):
            tile_of[b, s] = t
            t += int(NT[b, s])
    NTOT = t

    g_cols, g_off = 0, []
    for g, blocks in enumerate(groups):
        offs = []
        for s in range(2):
            ntg = int(sum(NT[b, s] for b in blocks))
            offs.append((g_cols, ntg))
            g_cols += ntg * 8
        g_off.append(offs)
    d_cols, d_off = 0, []
    for g, blocks in enumerate(groups):
        ntg = int(sum(NT[b, 0] + NT[b, 1] for b in blocks))
        d_off.append((d_cols, ntg))
        d_cols += ntg * 8

    plan = dict(N=N, NPC=NPC, NB=NB, SPLIT=SPLIT, NT=NT, groups=groups,
                tile_of=tile_of, NTOT=NTOT, g_off=g_off, d_off=d_off,
                g_cols=g_cols, d_cols=d_cols)

    per_core = []
    for c in range(NCORES):
        base = c * NPC
        gsegs, dsegs = [], []
        d_fp = np.full((NTOT, P), -1.0, np.float32)
        for g, blocks in enumerate(groups):
            for s in range(2):
                col0, ntg = g_off[g][s]
                idx = np.zeros(ntg * P, np.int64)
                pos = 0
                for b in blocks:
                    ss = lists[c][b][s][0]
                    nslots = int(NT[b, s]) * P
                    o2 = np.argsort(ss, kind="stable")  # src-ascending: HBM locality
                    idx[pos:pos + len(ss)] = ss[o2] - (SPLIT if s == 1 else 0)
                    pos += nslots
                gsegs.append((col0, idx.astype(np.int16)))
            col0, ntg = d_off[g]
            didx = np.zeros(ntg * P, np.int64)
            pos = 0
            for b in blocks:
                for s in range(2):
                    ss, dd = lists[c][b][s]
                    o2 = np.argsort(ss, kind="stable")
                    dd = dd[o2]
                    nslots = int(NT[b, s]) * P
                    didx[pos:pos + len(dd)] = dd - base
                    pos += nslots
                    t0 = int(tile_of[b, s])
                    dv = np.full(nslots, -1.0, np.float32)
                    dv[:len(dd)] = (dd - base - b * P).astype(np.float32)
                    d_fp[t0:t0 + int(NT[b, s])] = dv.reshape(int(NT[b, s]), P)
            dsegs.append((col0, didx.astype(np.int16)))
        per_core.append(dict(
            g_idx=_wrap_idx_segments(gsegs, g_cols),
            dl_idx=_wrap_idx_segments(dsegs, d_cols),
            d_fpT=np.ascontiguousarray(d_fp.T),   # [P, NTOT]
        ))
    return plan, per_core


def _build(plan, dims, has_b1, has_b2):
    import os
    V_ST = os.environ.get("GAT_V_ST", "merged")      # merged | base
    V_EXP = os.environ.get("GAT_V_EXP", "strided")   # strided | wbuf
    V_GATHER = os.environ.get("GAT_V_GATHER", "chunk8")  # merged | chunk8
    import concourse.bass as bass
    import concourse.bacc as bacc
    import concourse.tile as tile
    from concourse import mybir

    f32 = mybir.dt.float32
    bf16 = mybir.dt.bfloat16
    i16 = mybir.dt.int16
    AF = mybir.ActivationFunctionType
    OP = mybir.AluOpType

    N, NPC, NB, SPLIT = plan["N"], plan["NPC"], plan["NB"], plan["SPLIT"]
    NT, groups, tile_of, NTOT = (plan["NT"], plan["groups"], plan["tile_of"],
                                 plan["NTOT"])
    HID, H1, C1, OUT = dims["HID"], dims["H1"], dims["C1"], dims["OUT"]
    NPAD = ((N + P - 1) // P) * P
    NBA = NPAD // P
    ROW1 = 384                 # hext1 row stride (768B: mult of 256B)
    W1C = HID + 2 * H1         # 272 useful cols of hext1
    ROW2 = 128                 # 256B rows for score/L2 tables
    W2C = OUT + 2
    NBLK = NB * P
    NEG = 0.2

    nc = bacc.Bacc(num_devices=NCORES, num_swdge_queues=4)

    xT = nc.dram_tensor("xT", [2, P, NPAD], bf16, kind="ExternalInput")
    xTo = nc.dram_tensor("xTo", [2, P, NBLK], bf16, kind="ExternalInput")
    w1e = nc.dram_tensor("w1e", [2, P, W1C], bf16, kind="ExternalInput")
    w1s = nc.dram_tensor("w1s", [2, P, H1], bf16, kind="ExternalInput")
    w2e = nc.dram_tensor("w2e", [2, P, W2C], bf16, kind="ExternalInput")
    negcs = nc.dram_tensor("negcs", [P, W2C], f32, kind="ExternalInput")
    g_idx_d = nc.dram_tensor("g_idx", [P, plan["g_cols"]], i16, kind="ExternalInput")
    dl_idx_d = nc.dram_tensor("dl_idx", [P, plan["d_cols"]], i16, kind="ExternalInput")
    d_fpT_d = nc.dram_tensor("d_fpT", [P, NTOT], f32, kind="ExternalInput")
    if has_b1:
        b1_d = nc.dram_tensor("b1r", [P, HID], bf16, kind="ExternalInput")
    if has_b2:
        b2_d = nc.dram_tensor("b2r", [P, OUT], f32, kind="ExternalInput")
    out2 = nc.dram_tensor("out2", [NPC, OUT], f32, kind="ExternalOutput")

    hext1 = nc.dram_tensor("hext1", [NPAD + P, ROW1], bf16)
    s1dst = nc.dram_tensor("s1dst", [NBLK + P, ROW2], bf16)
    h2loc = nc.dram_tensor("h2loc", [NBLK + P, ROW2], bf16)
    h2full = nc.dram_tensor("h2full", [N + P, ROW2], bf16, addr_space="Shared")

    def sub_ap(t, elem_off, dims_):
        a = t[:]
        return bass.AP(tensor=a.tensor, offset=a.offset + elem_off,
                       ap=[a.ap[0]] + dims_)

    with tile.TileContext(nc, num_cores=NCORES) as tc:
        with tc.tile_pool(name="consts", bufs=1) as cp:
            w1t = []
            w1st = []
            w2t = []
            for k in range(2):
                t1 = cp.tile([P, W1C], bf16, tag=f"w1t{k}")
                nc.sync.dma_start(out=t1[:], in_=w1e[k])
                w1t.append(t1)
                t2 = cp.tile([P, H1], bf16, tag=f"w1st{k}")
                nc.sync.dma_start(out=t2[:], in_=w1s[k])
                w1st.append(t2)
                t3 = cp.tile([P, W2C], bf16, tag=f"w2t{k}")
                nc.sync.dma_start(out=t3[:], in_=w2e[k])
                w2t.append(t3)
            ncs_t = cp.tile([P, W2C], f32)
            nc.sync.dma_start(out=ncs_t[:], in_=negcs[:])
            gidx_t = cp.tile([P, plan["g_cols"]], i16)
            nc.sync.dma_start(out=gidx_t[:], in_=g_idx_d[:])
            dlidx_t = cp.tile([P, plan["d_cols"]], i16)
            nc.sync.dma_start(out=dlidx_t[:], in_=dl_idx_d[:])
            dfp_t = cp.tile([P, NTOT], f32)
            nc.sync.dma_start(out=dfp_t[:], in_=d_fpT_d[:])
            iota_i = cp.tile([P, P], mybir.dt.int32)
            nc.gpsimd.iota(iota_i[:], pattern=[[1, P]], base=0,
                           channel_multiplier=0)
            iota_t = cp.tile([P, P], bf16)
            nc.vector.tensor_copy(out=iota_t[:], in_=iota_i[:])
            iota_f = cp.tile([P, P], f32)
            nc.vector.tensor_copy(out=iota_f[:], in_=iota_i[:])
            dfp_b = cp.tile([P, NTOT], bf16)
            nc.vector.tensor_copy(out=dfp_b[:], in_=dfp_t[:])
            pidx_i = cp.tile([P, 1], mybir.dt.int32)
            nc.gpsimd.iota(pidx_i[:], pattern=[[0, 1]], base=0,
                           channel_multiplier=1)
            pidx_f = cp.tile([P, 1], f32)
            nc.vector.tensor_copy(out=pidx_f[:], in_=pidx_i[:])
            ident = cp.tile([P, P], bf16)
            nc.vector.tensor_scalar(out=ident[:], in0=iota_t[:], scalar1=pidx_f[:],
                                    scalar2=None, op0=OP.is_equal)
            b1_t = b2_t = None
            if has_b1:
                b1_t = cp.tile([P, HID], bf16)
                nc.sync.dma_start(out=b1_t[:], in_=b1_d[:])
            if has_b2:
                b2_t = cp.tile([P, OUT], f32)
                nc.sync.dma_start(out=b2_t[:], in_=b2_d[:])

            # ---------------- phase A: hext1 = [x@W1 | x@W1@A] ----------------
            CH = 8
            with (
                tc.tile_pool(name="xc", bufs=4) as xc,
                tc.tile_pool(name="psA", bufs=4, space="PSUM") as psA,
                tc.tile_pool(name="rowp", bufs=6) as rowp,
            ):
                for ch in range(0, NBA, CH):
                    ntc = min(CH, NBA - ch)
                    ck = []
                    for kh in range(2):
                        t_ = xc.tile([P, CH * P], bf16, tag="xchunk")
                        nc.sync.dma_start(out=t_[:, :ntc * P],
                                          in_=xT[kh, :, ch * P:(ch + ntc) * P])
                        ck.append(t_)
                    for j in range(ntc):
                        i = ch + j
                        ps = psA.tile([P, W1C], f32, tag="psA")
                        nc.tensor.matmul(ps[:], ck[0][:, j * P:(j + 1) * P],
                                         w1t[0][:], start=True, stop=False)
                        nc.tensor.matmul(ps[:], ck[1][:, j * P:(j + 1) * P],
                                         w1t[1][:], start=False, stop=True)
                        row = rowp.tile([P, W1C], bf16, tag="row")
                        if i % 2 == 0:
                            nc.scalar.activation(row[:], ps[:], AF.Copy)
                        else:
                            nc.vector.tensor_copy(out=row[:], in_=ps[:])
                        nc.sync.dma_start(out=hext1[i * P:(i + 1) * P, 0:W1C],
                                          in_=row[:])
                for ch in range(0, NB, CH):
                    ntc = min(CH, NB - ch)
                    ck = []
                    for kh in range(2):
                        t_ = xc.tile([P, CH * P], bf16, tag="xchunk2")
                        nc.sync.dma_start(out=t_[:, :ntc * P],
                                          in_=xTo[kh, :, ch * P:(ch + ntc) * P])
                        ck.append(t_)
                    for j in range(ntc):
                        i = ch + j
                        ps = psA.tile([P, H1], f32, tag="psA2")
                        nc.tensor.matmul(ps[:], ck[0][:, j * P:(j + 1) * P],
                                         w1st[0][:], start=True, stop=False)
                        nc.tensor.matmul(ps[:], ck[1][:, j * P:(j + 1) * P],
                                         w1st[1][:], start=False, stop=True)
                        row = rowp.tile([P, H1], bf16, tag="rows")
                        nc.vector.tensor_copy(out=row[:], in_=ps[:])
                        nc.sync.dma_start(out=s1dst[i * P:(i + 1) * P, 0:H1],
                                          in_=row[:])

            # ---------------- GAT conv layers ----------------
            def layer(lidx, table, nrows_tab, srow_g, tdst, H, F, sc_src, sc_dst,
                      epilogue):
                # gathered src row: [0:F feats | sc_src: H src scores]; the H
                # edge weights are written back over the src-score columns so
                # numerator and denominator come from ONE matmul per tile.
                MC = F + H
                qn = [0]
                with (
                    tc.tile_pool(name=f"g{lidx}", bufs=2) as gp,
                    tc.tile_pool(name=f"gd{lidx}", bufs=2) as gdp,
                    tc.tile_pool(name=f"wk{lidx}", bufs=3) as wk,
                    tc.tile_pool(name=f"st{lidx}", bufs=2) as stp,
                    tc.tile_pool(name=f"ps{lidx}", bufs=2, space="PSUM") as psp,
                    tc.tile_pool(name=f"pse{lidx}", bufs=2, space="PSUM") as pse,
                    tc.tile_pool(name=f"ep{lidx}", bufs=3) as ep,
                ):
                    for g, blocks in enumerate(groups):
                        gbuf = [None, None]
                        for s in range(2):
                            col0, ntg = plan["g_off"][g][s]
                            if ntg == 0:
                                continue
                            gt = gp.tile([P, ntg, srow_g], bf16, tag=f"g{s}")
                            base = 0 if s == 0 else SPLIT * ROW_TAB[lidx]
                            inap = bass.AP(
                                tensor=table[:].tensor, offset=base,
                                ap=[[ROW_TAB[lidx],
                                     nrows_tab - (0 if s == 0 else SPLIT)],
                                    [1, srow_g]])
                            cstep = ntg if V_GATHER == "merged" else 8
                            for c0 in range(0, ntg, cstep):
                                cn = min(cstep, ntg - c0)
                                nc.gpsimd.dma_gather(
                                    gt[:, c0:c0 + cn, :], inap,
                                    gidx_t[:, col0 + c0 * 8:col0 + (c0 + cn) * 8],
                                    cn * P, cn * P, srow_g,
                                    elem_step=ROW_TAB[lidx],
                                    single_packet=(cn <= 8),
                                    queue_num=qn[0] % 4)
                                qn[0] += 1
                            gbuf[s] = gt
                        dcol0, dntg = plan["d_off"][g]
                        gdt = gdp.tile([P, dntg, ROW2], bf16, tag="gd")
                        cstep = dntg if V_GATHER == "merged" else 8
                        for c0 in range(0, dntg, cstep):
                            cn = min(cstep, dntg - c0)
                            nc.gpsimd.dma_gather(
                                gdt[:, c0:c0 + cn, :], tdst[:],
                                dlidx_t[:, dcol0 + c0 * 8:dcol0 + (c0 + cn) * 8],
                                cn * P, cn * P, ROW2, elem_step=ROW2,
                                single_packet=(cn <= 8),
                                queue_num=qn[0] % 4)
                            qn[0] += 1

                        goff = [0, 0]
                        doff = 0
                        for b in blocks:
                            ntb = int(NT[b, 0] + NT[b, 1])
                            if ntb == 0:
                                continue
                            t0 = int(tile_of[b, 0])
                            # one-hot S_T for all tiles of the block
                            st = stp.tile([P, ntb, P], bf16, tag="st")
                            if V_ST == "merged":
                                in0 = sub_ap(iota_t, 0, [[0, ntb], [1, P]])
                                in1 = sub_ap(dfp_b, t0, [[1, ntb], [0, P]])
                                nc.vector.tensor_tensor(out=st[:], in0=in0,
                                                        in1=in1, op=OP.is_equal)
                            else:
                                for ti_ in range(ntb):
                                    nc.vector.tensor_scalar(
                                        out=st[:, ti_, :], in0=iota_t[:],
                                        scalar1=dfp_t[:, t0 + ti_:t0 + ti_ + 1],
                                        scalar2=None, op0=OP.is_equal)
                            # alpha = s_src[src] + s_dst[dst]  (f32)
                            al = wk.tile([P, ntb * H], f32, tag="al")
                            toff = 0
                            for s in range(2):
                                nts = int(NT[b, s])
                                if nts == 0:
                                    continue
                                gt = gbuf[s]
                                src_ap = sub_ap(gt, goff[s] * srow_g + sc_src,
                                                [[srow_g, nts], [1, H]])
                                dst_ap = sub_ap(gdt, (doff + toff) * ROW2 + sc_dst,
                                                [[ROW2, nts], [1, H]])
                                out_ap = sub_ap(al, toff * H,
                                                [[H, nts], [1, H]])
                                nc.vector.tensor_tensor(out=out_ap, in0=src_ap,
                                                        in1=dst_ap, op=OP.add)
                                toff += nts
                            # leaky relu in f32
                            nc.vector.scalar_tensor_tensor(
                                out=al[:], in0=al[:], scalar=NEG, in1=al[:],
                                op0=OP.mult, op1=OP.max)
                            # w = exp(alpha) written over the src-score cols,
                            # then weight the gathered features in place
                            if V_EXP == "wbuf":
                                wbuf = wk.tile([P, ntb * H], bf16, tag="wb")
                                nc.scalar.activation(wbuf[:], al[:], AF.Exp)
                            toff = 0
                            for s in range(2):
                                nts = int(NT[b, s])
                                if nts == 0:
                                    continue
                                gt = gbuf[s]
                                w_ap = sub_ap(gt, goff[s] * srow_g + sc_src,
                                              [[srow_g, nts], [1, H]])
                                al_ap = sub_ap(al, toff * H,
                                               [[1, nts * H]])
                                if V_EXP == "wbuf":
                                    wb_ap = sub_ap(wbuf, toff * H,
                                                   [[H, nts], [1, H]])
                                    nc.vector.tensor_copy(out=w_ap, in_=wb_ap)
                                else:
                                    nc.scalar.activation(w_ap, al_ap, AF.Exp)
                                if H > 1:
                                    gv = sub_ap(gt, goff[s] * srow_g,
                                                [[srow_g, nts], [F // H, H],
                                                 [1, F // H]])
                                    win = sub_ap(gt, goff[s] * srow_g + sc_src,
                                                 [[srow_g, nts], [1, H],
                                                  [0, F // H]])
                                else:
                                    gv = sub_ap(gt, goff[s] * srow_g,
                                                [[srow_g, nts], [1, F]])
                                    win = sub_ap(gt, goff[s] * srow_g + sc_src,
                                                 [[srow_g, nts], [0, F]])
                                nc.vector.tensor_tensor(out=gv, in0=gv, in1=win,
                                                        op=OP.mult)
                                toff += nts
                            # fused numerator+denominator matmul chain
                            ps = psp.tile([P, MC], f32, tag="num")
                            ti = 0
                            for s in range(2):
                                nts = int(NT[b, s])
                                gt = gbuf[s]
                                for j in range(nts):
                                    nc.tensor.matmul(
                                        ps[:], st[:, ti, :],
                                        gt[:, goff[s] + j, 0:MC],
                                        start=(ti == 0), stop=(ti == ntb - 1))
                                    ti += 1
                            rows = min(P, NPC - b * P)
                            epilogue(b, rows, ps, ep, pse)
                            goff[0] += int(NT[b, 0])
                            goff[1] += int(NT[b, 1])
                            doff += ntb

            def epi1(b, rows, ps, ep, pse):
                rden = ep.tile([P, H1], f32, tag="rden")
                nc.vector.reciprocal(rden[:], ps[:, HID:HID + H1])
                o = ep.tile([P, HID], bf16, tag="o")
                rb = sub_ap(rden, 0, [[1, H1], [0, C1]])
                num2 = bass.AP(tensor=ps[:].tensor, offset=ps[:].offset,
                               ap=[ps[:].ap[0], [C1, H1], [1, C1]])
                o2d = bass.AP(tensor=o[:].tensor, offset=o[:].offset,
                              ap=[o[:].ap[0], [C1, H1], [1, C1]])
                nc.vector.tensor_tensor(out=o2d, in0=num2, in1=rb, op=OP.mult)
                if b1_t is not None:
                    nc.vector.tensor_tensor(out=o[:], in0=o[:], in1=b1_t[:],
                                            op=OP.add)
                e = ep.tile([P, HID], bf16, tag="e")
                nc.scalar.activation(e[:], o[:], AF.Exp)
                nc.vector.tensor_scalar(out=o[:], in0=o[:], scalar1=0.0,
                                        scalar2=None, op0=OP.max)
                nc.vector.tensor_scalar(out=e[:], in0=e[:], scalar1=1.0,
                                        scalar2=None, op0=OP.min)
                nc.vector.tensor_tensor(out=o[:], in0=o[:], in1=e[:], op=OP.add)
                h2ps = pse.tile([P, W2C], f32, tag="h2ps")
                for half in range(2):
                    pt = pse.tile([P, P], bf16, tag="pt")
                    nc.tensor.transpose(pt[:], o[:, half * P:(half + 1) * P],
                                        ident[:])
                    et = ep.tile([P, P], bf16, tag="et")
                    nc.vector.tensor_copy(out=et[:], in_=pt[:])
                    nc.tensor.matmul(h2ps[:], et[:], w2t[half][:],
                                     start=(half == 0), stop=(half == 1))
                h2row = ep.tile([P, ROW2], bf16, tag="h2row")
                nc.vector.tensor_tensor(out=h2row[:, 0:W2C], in0=h2ps[:],
                                        in1=ncs_t[:], op=OP.add)
                nc.sync.dma_start(out=h2loc[b * P:b * P + P, :], in_=h2row[:])

            def epi2(b, rows, ps, ep, pse):
                rden = ep.tile([P, 1], f32, tag="rden2")
                nc.vector.reciprocal(rden[:], ps[:, OUT:OUT + 1])
                o = ep.tile([P, OUT], f32, tag="o2")
                nc.vector.tensor_scalar(out=o[:], in0=ps[:, 0:OUT],
                                        scalar1=rden[:],
                                        scalar2=None, op0=OP.mult)
                if b2_t is not None:
                    nc.vector.tensor_tensor(out=o[:], in0=o[:], in1=b2_t[:],
                                            op=OP.add)
                nc.sync.dma_start(out=out2[b * P:b * P + rows, :],
                                  in_=o[:rows, :])

            ROW_TAB = {1: ROW1, 2: ROW2}
            layer(1, hext1, NPAD, ROW1, s1dst, H1, HID, HID, 0, epi1)
            nc.gpsimd.collective_compute(
                "AllGather", mybir.AluOpType.bypass,
                replica_groups=[list(range(NCORES))],
                ins=[h2loc[0:NPC, :]], outs=[h2full[0:N, :]],
            )
            layer(2, h2full, N, ROW2, h2loc, 1, OUT, OUT, OUT + 1, epi2)

    nc.finalize()
    return nc


def _host_prep_weights(W1, att1, W2, att2):
    HID = W1.shape[1]
    H1 = att1.shape[1]
    C1 = HID // H1
    OUT = W2.shape[1]
    A_src = np.zeros((HID, H1), np.float32)
    A_dst = np.zeros((HID, H1), np.float32)
    for h in range(H1):
        A_src[h * C1:(h + 1) * C1, h] = att1[0, h, C1:]
        A_dst[h * C1:(h + 1) * C1, h] = att1[0, h, :C1]
    W1ext = np.concatenate([W1, W1 @ A_src, W1 @ A_dst], axis=1)
    W1sco = W1 @ A_dst
    a2 = att2[0, 0]
    W2ext = np.concatenate([W2, (W2 @ a2[OUT:])[:, None],
                            (W2 @ a2[:OUT])[:, None]], axis=1)
    return W1ext, W1sco, W2ext


def kernel(x, edge_index, W1, att1, b1, W2, att2, b2):
    import os
    from concourse import mybir
    from concourse.bass_utils import run_bass_kernel_spmd
    ml_bf16 = mybir.dt.np(mybir.dt.bfloat16)

    x = np.asarray(x, np.float32)
    edge_index = np.asarray(edge_index)
    W1 = np.asarray(W1, np.float32)
    att1 = np.asarray(att1, np.float32)
    b1 = np.asarray(b1, np.float32)
    W2 = np.asarray(W2, np.float32)
    att2 = np.asarray(att2, np.float32)
    b2 = np.asarray(b2, np.float32)

    N, IN = x.shape
    HID = W1.shape[1]
    H1 = att1.shape[1]
    C1 = HID // H1
    OUT = W2.shape[1]
    NPC = N // NCORES
    NB = (NPC + P - 1) // P
    NPAD = ((N + P - 1) // P) * P
    NBLK = NB * P

    plan, per_core = _prep(x, edge_index)
    dims = dict(IN=IN, HID=HID, H1=H1, C1=C1, OUT=OUT)
    has_b1 = bool(np.any(b1 != 0))
    has_b2 = bool(np.any(b2 != 0))

    key = (N, IN, HID, H1, OUT, plan["g_cols"], plan["d_cols"], plan["NTOT"],
           has_b1, has_b2, tuple(int(v) for v in plan["NT"].ravel()),
           os.environ.get("GAT_V_ST"), os.environ.get("GAT_V_EXP"),
           os.environ.get("GAT_V_GATHER"))
    if key not in _CACHE:
        _CACHE[key] = _build(plan, dims, has_b1, has_b2)
    nc = _CACHE[key]

    W1ext, W1sco, W2ext = _host_prep_weights(W1, att1, W2, att2)
    negcs = np.tile(-W2ext.sum(axis=0, keepdims=True), (P, 1)).astype(np.float32)

    xTfull = np.zeros((IN, NPAD), np.float32)
    xTfull[:, :N] = x.T
    xT = xTfull.reshape(2, P, NPAD).astype(ml_bf16)

    def ktiles(w):
        return np.ascontiguousarray(w.reshape(2, P, -1)).astype(ml_bf16)

    in_maps = []
    for c in range(NCORES):
        xo = np.zeros((IN, NBLK), np.float32)
        xo[:, :NPC] = x[c * NPC:(c + 1) * NPC].T
        m = dict(
            xT=xT,
            xTo=xo.reshape(2, P, NBLK).astype(ml_bf16),
            w1e=ktiles(W1ext),
            w1s=ktiles(W1sco),
            w2e=ktiles(W2ext),
            negcs=negcs,
            g_idx=per_core[c]["g_idx"],
            dl_idx=per_core[c]["dl_idx"],
            d_fpT=per_core[c]["d_fpT"],
        )
        if has_b1:
            m["b1r"] = np.tile(b1[None, :], (P, 1)).astype(ml_bf16)
        if has_b2:
            m["b2r"] = np.tile(b2[None, :], (P, 1)).astype(np.float32)
        in_maps.append(m)

    trace = bool(os.environ.get("GAT_TRACE"))
    kw = {}
    if trace:
        kw = dict(trace=True,
                  tmpdir=os.environ.get("GAT_TRACE_DIR") or None,
                  trace_cores=[int(c) for c in
                               os.environ.get("GAT_TRACE_CORES", "0").split(",")])
    res = run_bass_kernel_spmd(nc, in_maps, list(range(NCORES)), **kw)
    globals()["LAST_RESULT"] = res
    out = np.concatenate([res.results[c]["out2"] for c in range(NCORES)], axis=0)
    return np.ascontiguousarray(out.astype(np.float32))


# revision 16
# speedup vs baseline: 1.7191x; 1.0702x over previous
"""2-layer GAT on 8 Trainium2 NeuronCores (Bass/Tile).

Sharding: edges sorted by destination node; nodes partitioned 8 x N/8 across
cores (dst-partitioned edge-parallel). Per-dst softmax groups stay entirely on
one core, so aggregation needs no cross-core reduction. Per 128-node block:
dma_gather of per-edge source feature rows from a replicated table (two halves
so indices fit int16), one-hot S_T built via is_equal, and the weighted
scatter-add + softmax denominator computed as ONE PSUM-accumulated bf16 matmul
per 128-edge tile (weights written into the gathered rows' score columns).
One AllGather shares the small layer-2 feature table between the layers.
"""
import numpy as np

P = 128
NCORES = 8

_CACHE = {}


def _wrap_idx_segments(segs, total_cols):
    arr = np.zeros((16, total_cols), np.int16)
    for off, idx in segs:
        n = len(idx)
        if n:
            arr[:, off:off + n // 16] = idx.reshape(n // 16, 16).T
    return np.tile(arr, (8, 1))


def _prep(x, edge_index):
    N = x.shape[0]
    NPC = N // NCORES
    NB = (NPC + P - 1) // P
    SPLIT = N // 2

    src = np.concatenate([np.asarray(edge_index[0]), np.arange(N, dtype=np.int64)])
    dst = np.concatenate([np.asarray(edge_index[1]), np.arange(N, dtype=np.int64)])
    order = np.argsort(dst, kind="stable")
    s_all = src[order].astype(np.int64)
    d_all = dst[order].astype(np.int64)

    lists = [[[None, None] for _ in range(NB)] for _ in range(NCORES)]
    for c in range(NCORES):
        base = c * NPC
        for b in range(NB):
            e0 = np.searchsorted(d_all, base + b * P)
            e1 = np.searchsorted(d_all, min(base + (b + 1) * P, base + NPC))
            ss, dd = s_all[e0:e1], d_all[e0:e1]
            m = ss < SPLIT
            lists[c][b][0] = (ss[m], dd[m])
            lists[c][b][1] = (ss[~m], dd[~m])

    NT = np.zeros((NB, 2), np.int64)
    for b in range(NB):
        for s in range(2):
            mx = max(len(lists[c][b][s][0]) for c in range(NCORES))
            NT[b, s] = (mx + P - 1) // P

    groups = [list(range(g, min(g + 2, NB))) for g in range(0, NB, 2)]

    tile_of = np.zeros((NB, 2), np.int64)
    t = 0
    for b in range(NB):
        for s in range(2):
            tile_of[b, s] = t
            t += int(NT[b, s])
    NTOT = t

    g_cols, g_off = 0, []
    for g, blocks in enumerate(groups):
        offs = []
        for s in range(2):
            ntg = int(sum(NT[b, s] for b in blocks))
            offs.append((g_cols, ntg))
            g_cols += ntg * 8
        g_off.append(offs)
    d_cols, d_off = 0, []
    for g, blocks in enumerate(groups):
        ntg = int(sum(NT[b, 0] + NT[b, 1] for b in blocks))
        d_off.append((d_cols, ntg))
        d_cols += ntg * 8

    plan = dict(N=N, NPC=NPC, NB=NB, SPLIT=SPLIT, NT=NT, groups=groups,
                tile_of=tile_of, NTOT=NTOT, g_off=g_off, d_off=d_off,
                g_cols=g_cols, d_cols=d_cols)

    per_core = []
    for c in range(NCORES):
        base = c * NPC
        gsegs, dsegs = [], []
        d_fp = np.full((NTOT, P), -1.0, np.float32)
        for g, blocks in enumerate(groups):
            for s in range(2):
                col0, ntg = g_off[g][s]
                idx = np.zeros(ntg * P, np.int64)
                pos = 0
                for b in blocks:
                    ss = lists[c][b][s][0]
                    nslots = int(NT[b, s]) * P
                    o2 = np.argsort(ss, kind="stable")  # src-ascending: HBM locality
                    idx[pos:pos + len(ss)] = ss[o2] - (SPLIT if s == 1 else 0)
                    pos += nslots
                gsegs.append((col0, idx.astype(np.int16)))
            col0, ntg = d_off[g]
            didx = np.zeros(ntg * P, np.int64)
            pos = 0
            for b in blocks:
                for s in range(2):
                    ss, dd = lists[c][b][s]
                    o2 = np.argsort(ss, kind="stable")
                    dd = dd[o2]
                    nslots = int(NT[b, s]) * P
                    didx[pos:pos + len(dd)] = dd - base
                    pos += nslots
                    t0 = int(tile_of[b, s])
                    dv = np.full(nslots, -1.0, np.float32)
                    dv[:len(dd)] = (dd - base - b * P).astype(np.float32)
                    d_fp[t0:t0 + int(NT[b, s])] = dv.reshape(int(NT[b, s]), P)
            dsegs.append((col0, didx.astype(np.int16)))
        per_core.append(dict(
            g_idx=_wrap_idx_segments(gsegs, g_cols),
            dl_idx=_wrap_idx_segments(dsegs, d_cols),
            d_fpT=np.ascontiguousarray(d_fp.T),   # [P, NTOT]
        ))
    return plan, per_core


def _build(plan, dims, has_b1, has_b2):
    import os
    V_ST = os.environ.get("GAT_V_ST", "merged")      # merged | base
    V_EXP = os.environ.get("GAT_V_EXP", "strided")   # strided | wbuf
    V_GATHER = os.environ.get("GAT_V_GATHER", "chunk8")  # merged | chunk8
    import concourse.bass as bass
    import concourse.bacc as bacc
    import concourse.tile as tile
    from concourse import mybir

    f32 = mybir.dt.float32
    bf16 = mybir.dt.bfloat16
    i16 = mybir.dt.int16
    AF = mybir.ActivationFunctionType
    OP = mybir.AluOpType

    N, NPC, NB, SPLIT = plan["N"], plan["NPC"], plan["NB"], plan["SPLIT"]
    NT, groups, tile_of, NTOT = (plan["NT"], plan["groups"], plan["tile_of"],
                                 plan["NTOT"])
    HID, H1, C1, OUT = dims["HID"], dims["H1"], dims["C1"], dims["OUT"]
    NPAD = ((N + P - 1) // P) * P
    NBA = NPAD // P
    ROW1 = 384                 # hext1 row stride (768B: mult of 256B)
    W1C = HID + 2 * H1         # 272 useful cols of hext1
    ROW2 = 128                 # 256B rows for score/L2 tables
    W2C = OUT + 2
    NBLK = NB * P
    NEG = 0.2

    nc = bacc.Bacc(num_devices=NCORES, num_swdge_queues=4)

    xT = nc.dram_tensor("xT", [2, P, NPAD], bf16, kind="ExternalInput")
    xTo = nc.dram_tensor("xTo", [2, P, NBLK], bf16, kind="ExternalInput")
    w1e = nc.dram_tensor("w1e", [2, P, W1C], bf16, kind="ExternalInput")
    w1s = nc.dram_tensor("w1s", [2, P, H1], bf16, kind="ExternalInput")
    w2e = nc.dram_tensor("w2e", [2, P, W2C], bf16, kind="ExternalInput")
    negcs = nc.dram_tensor("negcs", [P, W2C], f32, kind="ExternalInput")
    g_idx_d = nc.dram_tensor("g_idx", [P, plan["g_cols"]], i16, kind="ExternalInput")
    dl_idx_d = nc.dram_tensor("dl_idx", [P, plan["d_cols"]], i16, kind="ExternalInput")
    d_fpT_d = nc.dram_tensor("d_fpT", [P, NTOT], f32, kind="ExternalInput")
    if has_b1:
        b1_d = nc.dram_tensor("b1r", [P, HID], bf16, kind="ExternalInput")
    if has_b2:
        b2_d = nc.dram_tensor("b2r", [P, OUT], f32, kind="ExternalInput")
    out2 = nc.dram_tensor("out2", [NPC, OUT], f32, kind="ExternalOutput")

    hext1 = nc.dram_tensor("hext1", [NPAD + P, ROW1], bf16)
    s1dst = nc.dram_tensor("s1dst", [NBLK + P, ROW2], bf16)
    h2loc = nc.dram_tensor("h2loc", [NBLK + P, ROW2], bf16)
    h2full = nc.dram_tensor("h2full", [N + P, ROW2], bf16, addr_space="Shared")

    def sub_ap(t, elem_off, dims_):
        a = t[:]
        return bass.AP(tensor=a.tensor, offset=a.offset + elem_off,
                       ap=[a.ap[0]] + dims_)

    with tile.TileContext(nc, num_cores=NCORES) as tc:
        with tc.tile_pool(name="consts", bufs=1) as cp:
            w1t = []
            w1st = []
            w2t = []
            for k in range(2):
                t1 = cp.tile([P, W1C], bf16, tag=f"w1t{k}")
                nc.sync.dma_start(out=t1[:], in_=w1e[k])
                w1t.append(t1)
                t2 = cp.tile([P, H1], bf16, tag=f"w1st{k}")
                nc.sync.dma_start(out=t2[:], in_=w1s[k])
                w1st.append(t2)
                t3 = cp.tile([P, W2C], bf16, tag=f"w2t{k}")
                nc.sync.dma_start(out=t3[:], in_=w2e[k])
                w2t.append(t3)
            ncs_t = cp.tile([P, W2C], f32)
            nc.sync.dma_start(out=ncs_t[:], in_=negcs[:])
            gidx_t = cp.tile([P, plan["g_cols"]], i16)
            nc.sync.dma_start(out=gidx_t[:], in_=g_idx_d[:])
            dlidx_t = cp.tile([P, plan["d_cols"]], i16)
            nc.sync.dma_start(out=dlidx_t[:], in_=dl_idx_d[:])
            dfp_t = cp.tile([P, NTOT], f32)
            nc.sync.dma_start(out=dfp_t[:], in_=d_fpT_d[:])
            iota_i = cp.tile([P, P], mybir.dt.int32)
            nc.gpsimd.iota(iota_i[:], pattern=[[1, P]], base=0,
                           channel_multiplier=0)
            iota_t = cp.tile([P, P], bf16)
            nc.vector.tensor_copy(out=iota_t[:], in_=iota_i[:])
            iota_f = cp.tile([P, P], f32)
            nc.vector.tensor_copy(out=iota_f[:], in_=iota_i[:])
            dfp_b = cp.tile([P, NTOT], bf16)
            nc.vector.tensor_copy(out=dfp_b[:], in_=dfp_t[:])
            pidx_i = cp.tile([P, 1], mybir.dt.int32)
            nc.gpsimd.iota(pidx_i[:], pattern=[[0, 1]], base=0,
                           channel_multiplier=1)
            pidx_f = cp.tile([P, 1], f32)
            nc.vector.tensor_copy(out=pidx_f[:], in_=pidx_i[:])
            ident = cp.tile([P, P], bf16)
            nc.vector.tensor_scalar(out=ident[:], in0=iota_t[:], scalar1=pidx_f[:],
                                    scalar2=None, op0=OP.is_equal)
            b1_t = b2_t = None
            if has_b1:
                b1_t = cp.tile([P, HID], bf16)
                nc.sync.dma_start(out=b1_t[:], in_=b1_d[:])
            if has_b2:
                b2_t = cp.tile([P, OUT], f32)
                nc.sync.dma_start(out=b2_t[:], in_=b2_d[:])

            # ---------------- phase A: hext1 = [x@W1 | x@W1@A] ----------------
            CH = 8
            with (
                tc.tile_pool(name="xc", bufs=4) as xc,
                tc.tile_pool(name="psA", bufs=4, space="PSUM") as psA,
                tc.tile_pool(name="rowp", bufs=6) as rowp,
            ):
                for ch in range(0, NBA, CH):
                    ntc = min(CH, NBA - ch)
                    ck = []
                    for kh in range(2):
                        t_ = xc.tile([P, CH * P], bf16, tag="xchunk")
                        nc.sync.dma_start(out=t_[:, :ntc * P],
                                          in_=xT[kh, :, ch * P:(ch + ntc) * P])
                        ck.append(t_)
                    for j in range(ntc):
                        i = ch + j
                        ps = psA.tile([P, W1C], f32, tag="psA")
                        nc.tensor.matmul(ps[:], ck[0][:, j * P:(j + 1) * P],
                                         w1t[0][:], start=True, stop=False)
                        nc.tensor.matmul(ps[:], ck[1][:, j * P:(j + 1) * P],
                                         w1t[1][:], start=False, stop=True)
                        row = rowp.tile([P, W1C], bf16, tag="row")
                        if i % 2 == 0:
                            nc.scalar.activation(row[:], ps[:], AF.Copy)
                        else:
                            nc.vector.tensor_copy(out=row[:], in_=ps[:])
                        nc.sync.dma_start(out=hext1[i * P:(i + 1) * P, 0:W1C],
                                          in_=row[:])
                for ch in range(0, NB, CH):
                    ntc = min(CH, NB - ch)
                    ck = []
                    for kh in range(2):
                        t_ = xc.tile([P, CH * P], bf16, tag="xchunk2")
                        nc.sync.dma_start(out=t_[:, :ntc * P],
                                          in_=xTo[kh, :, ch * P:(ch + ntc) * P])
                        ck.append(t_)
                    for j in range(ntc):
                        i = ch + j
                        ps = psA.tile([P, H1], f32, tag="psA2")
                        nc.tensor.matmul(ps[:], ck[0][:, j * P:(j + 1) * P],
                                         w1st[0][:], start=True, stop=False)
                        nc.tensor.matmul(ps[:], ck[1][:, j * P:(j + 1) * P],
                                         w1st[1][:], start=False, stop=True)
                        row = rowp.tile([P, H1], bf16, tag="rows")
                        nc.vector.tensor_copy(out=row[:], in_=ps[:])
                        nc.sync.dma_start(out=s1dst[i * P:(i + 1) * P, 0:H1],
                                          in_=row[:])

            # ---------------- GAT conv layers ----------------
            def layer(lidx, table, nrows_tab, srow_g, tdst, H, F, sc_src, sc_dst,
                      epilogue):
                # gathered src row: [0:F feats | sc_src: H src scores]; the H
                # edge weights are written back over the src-score columns so
                # numerator and denominator come from ONE matmul per tile.
                MC = F + H
                qn = [0]
                with (
                    tc.tile_pool(name=f"g{lidx}", bufs=3) as gp,
                    tc.tile_pool(name=f"gd{lidx}", bufs=3) as gdp,
                    tc.tile_pool(name=f"wk{lidx}", bufs=3) as wk,
                    tc.tile_pool(name=f"st{lidx}", bufs=2) as stp,
                    tc.tile_pool(name=f"ps{lidx}", bufs=2, space="PSUM") as psp,
                    tc.tile_pool(name=f"pse{lidx}", bufs=2, space="PSUM") as pse,
                    tc.tile_pool(name=f"ep{lidx}", bufs=3) as ep,
                ):
                    for g, blocks in enumerate(groups):
                        gbuf = [None, None]
                        for s in range(2):
                            col0, ntg = plan["g_off"][g][s]
                            if ntg == 0:
                                continue
                            gt = gp.tile([P, ntg, srow_g], bf16, tag=f"g{s}")
                            base = 0 if s == 0 else SPLIT * ROW_TAB[lidx]
                            inap = bass.AP(
                                tensor=table[:].tensor, offset=base,
                                ap=[[ROW_TAB[lidx],
                                     nrows_tab - (0 if s == 0 else SPLIT)],
                                    [1, srow_g]])
                            cstep = ntg if V_GATHER == "merged" else 8
                            for c0 in range(0, ntg, cstep):
                                cn = min(cstep, ntg - c0)
                                nc.gpsimd.dma_gather(
                                    gt[:, c0:c0 + cn, :], inap,
                                    gidx_t[:, col0 + c0 * 8:col0 + (c0 + cn) * 8],
                                    cn * P, cn * P, srow_g,
                                    elem_step=ROW_TAB[lidx],
                                    single_packet=(cn <= 8),
                                    queue_num=qn[0] % 4)
                                qn[0] += 1
                            gbuf[s] = gt
                        dcol0, dntg = plan["d_off"][g]
                        gdt = gdp.tile([P, dntg, ROW2], bf16, tag="gd")
                        cstep = dntg if V_GATHER == "merged" else 8
                        for c0 in range(0, dntg, cstep):
                            cn = min(cstep, dntg - c0)
                            nc.gpsimd.dma_gather(
                                gdt[:, c0:c0 + cn, :], tdst[:],
                                dlidx_t[:, dcol0 + c0 * 8:dcol0 + (c0 + cn) * 8],
                                cn * P, cn * P, ROW2, elem_step=ROW2,
                                single_packet=(cn <= 8),
                                queue_num=qn[0] % 4)
                            qn[0] += 1

                        goff = [0, 0]
                        doff = 0
                        for b in blocks:
                            ntb = int(NT[b, 0] + NT[b, 1])
                            if ntb == 0:
                                continue
                            t0 = int(tile_of[b, 0])
                            # one-hot S_T for all tiles of the block
                            st = stp.tile([P, ntb, P], bf16, tag="st")
                            if V_ST == "merged":
                                in0 = sub_ap(iota_t, 0, [[0, ntb], [1, P]])
                                in1 = sub_ap(dfp_b, t0, [[1, ntb], [0, P]])
                                nc.vector.tensor_tensor(out=st[:], in0=in0,
                                                        in1=in1, op=OP.is_equal)
                            else:
                                for ti_ in range(ntb):
                                    nc.vector.tensor_scalar(
                                        out=st[:, ti_, :], in0=iota_t[:],
                                        scalar1=dfp_t[:, t0 + ti_:t0 + ti_ + 1],
                                        scalar2=None, op0=OP.is_equal)
                            # alpha = s_src[src] + s_dst[dst]  (f32)
                            al = wk.tile([P, ntb * H], f32, tag="al")
                            toff = 0
                            for s in range(2):
                                nts = int(NT[b, s])
                                if nts == 0:
                                    continue
                                gt = gbuf[s]
                                src_ap = sub_ap(gt, goff[s] * srow_g + sc_src,
                                                [[srow_g, nts], [1, H]])
                                dst_ap = sub_ap(gdt, (doff + toff) * ROW2 + sc_dst,
                                                [[ROW2, nts], [1, H]])
                                out_ap = sub_ap(al, toff * H,
                                                [[H, nts], [1, H]])
                                nc.vector.tensor_tensor(out=out_ap, in0=src_ap,
                                                        in1=dst_ap, op=OP.add)
                                toff += nts
                            # leaky relu in f32
                            nc.vector.scalar_tensor_tensor(
                                out=al[:], in0=al[:], scalar=NEG, in1=al[:],
                                op0=OP.mult, op1=OP.max)
                            # w = exp(alpha) written over the src-score cols,
                            # then weight the gathered features in place
                            if V_EXP == "wbuf":
                                wbuf = wk.tile([P, ntb * H], bf16, tag="wb")
                                nc.scalar.activation(wbuf[:], al[:], AF.Exp)
                            toff = 0
                            for s in range(2):
                                nts = int(NT[b, s])
                                if nts == 0:
                                    continue
                                gt = gbuf[s]
                                w_ap = sub_ap(gt, goff[s] * srow_g + sc_src,
                                              [[srow_g, nts], [1, H]])
                                al_ap = sub_ap(al, toff * H,
                                               [[1, nts * H]])
                                if V_EXP == "wbuf":
                                    wb_ap = sub_ap(wbuf, toff * H,
                                                   [[H, nts], [1, H]])
                                    nc.vector.tensor_copy(out=w_ap, in_=wb_ap)
                                else:
                                    nc.scalar.activation(w_ap, al_ap, AF.Exp)
                                if H > 1:
                                    gv = sub_ap(gt, goff[s] * srow_g,
                                                [[srow_g, nts], [F // H, H],
                                                 [1, F // H]])
                                    win = sub_ap(gt, goff[s] * srow_g + sc_src,
                                                 [[srow_g, nts], [1, H],
                                                  [0, F // H]])
                                else:
                                    gv = sub_ap(gt, goff[s] * srow_g,
                                                [[srow_g, nts], [1, F]])
                                    win = sub_ap(gt, goff[s] * srow_g + sc_src,
                                                 [[srow_g, nts], [0, F]])
                                nc.vector.tensor_tensor(out=gv, in0=gv, in1=win,
                                                        op=OP.mult)
                                toff += nts
                            # fused numerator+denominator matmul chain
                            ps = psp.tile([P, MC], f32, tag="num")
                            ti = 0
                            for s in range(2):
                                nts = int(NT[b, s])
                                gt = gbuf[s]
                                for j in range(nts):
                                    nc.tensor.matmul(
                                        ps[:], st[:, ti, :],
                                        gt[:, goff[s] + j, 0:MC],
                                        start=(ti == 0), stop=(ti == ntb - 1))
                                    ti += 1
                            rows = min(P, NPC - b * P)
                            epilogue(b, rows, ps, ep, pse)
                            goff[0] += int(NT[b, 0])
                            goff[1] += int(NT[b, 1])
                            doff += ntb

            def epi1(b, rows, ps, ep, pse):
                rden = ep.tile([P, H1], f32, tag="rden")
                nc.vector.reciprocal(rden[:], ps[:, HID:HID + H1])
                o = ep.tile([P, HID], bf16, tag="o")
                rb = sub_ap(rden, 0, [[1, H1], [0, C1]])
                num2 = bass.AP(tensor=ps[:].tensor, offset=ps[:].offset,
                               ap=[ps[:].ap[0], [C1, H1], [1, C1]])
                o2d = bass.AP(tensor=o[:].tensor, offset=o[:].offset,
                              ap=[o[:].ap[0], [C1, H1], [1, C1]])
                nc.vector.tensor_tensor(out=o2d, in0=num2, in1=rb, op=OP.mult)
                if b1_t is not None:
                    nc.vector.tensor_tensor(out=o[:], in0=o[:], in1=b1_t[:],
                                            op=OP.add)
                e = ep.tile([P, HID], bf16, tag="e")
                nc.scalar.activation(e[:], o[:], AF.Exp)
                nc.vector.tensor_scalar(out=o[:], in0=o[:], scalar1=0.0,
                                        scalar2=None, op0=OP.max)
                nc.vector.tensor_scalar(out=e[:], in0=e[:], scalar1=1.0,
                                        scalar2=None, op0=OP.min)
                nc.vector.tensor_tensor(out=o[:], in0=o[:], in1=e[:], op=OP.add)
                h2ps = pse.tile([P, W2C], f32, tag="h2ps")
                for half in range(2):
                    pt = pse.tile([P, P], bf16, tag="pt")
                    nc.tensor.transpose(pt[:], o[:, half * P:(half + 1) * P],
                                        ident[:])
                    et = ep.tile([P, P], bf16, tag="et")
                    nc.vector.tensor_copy(out=et[:], in_=pt[:])
                    nc.tensor.matmul(h2ps[:], et[:], w2t[half][:],
                                     start=(half == 0), stop=(half == 1))
                h2row = ep.tile([P, ROW2], bf16, tag="h2row")
                nc.vector.tensor_tensor(out=h2row[:, 0:W2C], in0=h2ps[:],
                                        in1=ncs_t[:], op=OP.add)
                nc.sync.dma_start(out=h2loc[b * P:b * P + P, :], in_=h2row[:])

            def epi2(b, rows, ps, ep, pse):
                rden = ep.tile([P, 1], f32, tag="rden2")
                nc.vector.reciprocal(rden[:], ps[:, OUT:OUT + 1])
                o = ep.tile([P, OUT], f32, tag="o2")
                nc.vector.tensor_scalar(out=o[:], in0=ps[:, 0:OUT],
                                        scalar1=rden[:],
                                        scalar2=None, op0=OP.mult)
                if b2_t is not None:
                    nc.vector.tensor_tensor(out=o[:], in0=o[:], in1=b2_t[:],
                                            op=OP.add)
                nc.sync.dma_start(out=out2[b * P:b * P + rows, :],
                                  in_=o[:rows, :])

            ROW_TAB = {1: ROW1, 2: ROW2}
            layer(1, hext1, NPAD, ROW1, s1dst, H1, HID, HID, 0, epi1)
            nc.gpsimd.collective_compute(
                "AllGather", mybir.AluOpType.bypass,
                replica_groups=[list(range(NCORES))],
                ins=[h2loc[0:NPC, :]], outs=[h2full[0:N, :]],
            )
            layer(2, h2full, N, ROW2, h2loc, 1, OUT, OUT, OUT + 1, epi2)

    nc.finalize()
    return nc


def _host_prep_weights(W1, att1, W2, att2):
    HID = W1.shape[1]
    H1 = att1.shape[1]
    C1 = HID // H1
    OUT = W2.shape[1]
    A_src = np.zeros((HID, H1), np.float32)
    A_dst = np.zeros((HID, H1), np.float32)
    for h in range(H1):
        A_src[h * C1:(h + 1) * C1, h] = att1[0, h, C1:]
        A_dst[h * C1:(h + 1) * C1, h] = att1[0, h, :C1]
    W1ext = np.concatenate([W1, W1 @ A_src, W1 @ A_dst], axis=1)
    W1sco = W1 @ A_dst
    a2 = att2[0, 0]
    W2ext = np.concatenate([W2, (W2 @ a2[OUT:])[:, None],
                            (W2 @ a2[:OUT])[:, None]], axis=1)
    return W1ext, W1sco, W2ext


def kernel(x, edge_index, W1, att1, b1, W2, att2, b2):
    import os
    from concourse import mybir
    from concourse.bass_utils import run_bass_kernel_spmd
    ml_bf16 = mybir.dt.np(mybir.dt.bfloat16)

    x = np.asarray(x, np.float32)
    edge_index = np.asarray(edge_index)
    W1 = np.asarray(W1, np.float32)
    att1 = np.asarray(att1, np.float32)
    b1 = np.asarray(b1, np.float32)
    W2 = np.asarray(W2, np.float32)
    att2 = np.asarray(att2, np.float32)
    b2 = np.asarray(b2, np.float32)

    N, IN = x.shape
    HID = W1.shape[1]
    H1 = att1.shape[1]
    C1 = HID // H1
    OUT = W2.shape[1]
    NPC = N // NCORES
    NB = (NPC + P - 1) // P
    NPAD = ((N + P - 1) // P) * P
    NBLK = NB * P

    plan, per_core = _prep(x, edge_index)
    dims = dict(IN=IN, HID=HID, H1=H1, C1=C1, OUT=OUT)
    has_b1 = bool(np.any(b1 != 0))
    has_b2 = bool(np.any(b2 != 0))

    key = (N, IN, HID, H1, OUT, plan["g_cols"], plan["d_cols"], plan["NTOT"],
           has_b1, has_b2, tuple(int(v) for v in plan["NT"].ravel()),
           os.environ.get("GAT_V_ST"), os.environ.get("GAT_V_EXP"),
           os.environ.get("GAT_V_GATHER"))
    if key not in _CACHE:
        _CACHE[key] = _build(plan, dims, has_b1, has_b2)
    nc = _CACHE[key]

    W1ext, W1sco, W2ext = _host_prep_weights(W1, att1, W2, att2)
    negcs = np.tile(-W2ext.sum(axis=0, keepdims=True), (P, 1)).astype(np.float32)

    xTfull = np.zeros((IN, NPAD), np.float32)
    xTfull[:, :N] = x.T
    xT = xTfull.reshape(2, P, NPAD).astype(ml_bf16)

    def ktiles(w):
        return np.ascontiguousarray(w.reshape(2, P, -1)).astype(ml_bf16)

    in_maps = []
    for c in range(NCORES):
        xo = np.zeros((IN, NBLK), np.float32)
        xo[:, :NPC] = x[c * NPC:(c + 1) * NPC].T
        m = dict(
            xT=xT,
            xTo=xo.reshape(2, P, NBLK).astype(ml_bf16),
            w1e=ktiles(W1ext),
            w1s=ktiles(W1sco),
            w2e=ktiles(W2ext),
            negcs=negcs,
            g_idx=per_core[c]["g_idx"],
            dl_idx=per_core[c]["dl_idx"],
            d_fpT=per_core[c]["d_fpT"],
        )
        if has_b1:
            m["b1r"] = np.tile(b1[None, :], (P, 1)).astype(ml_bf16)
        if has_b2:
            m["b2r"] = np.tile(b2[None, :], (P, 1)).astype(np.float32)
        in_maps.append(m)

    trace = bool(os.environ.get("GAT_TRACE"))
    kw = {}
    if trace:
        kw = dict(trace=True,
                  tmpdir=os.environ.get("GAT_TRACE_DIR") or None,
                  trace_cores=[int(c) for c in
                               os.environ.get("GAT_TRACE_CORES", "0").split(",")])
    res = run_bass_kernel_spmd(nc, in_maps, list(range(NCORES)), **kw)
    globals()["LAST_RESULT"] = res
    out = np.concatenate([res.results[c]["out2"] for c in range(NCORES)], axis=0)
    return np.ascontiguousarray(out.astype(np.float32))
